# revision 24
# baseline (speedup 1.0000x reference)
"""Trainium2 Bass kernel for nn_PositionalEncoding.

The reference module's realized math discards the input entirely:
    out = broadcast_to(2 * pe_table, x.shape)   # pe_table is [64, 16]
so the kernel is pure output-bandwidth bound. Each of the 8 cores writes
its (8192, 64, 16) f32 shard (32 MiB) of the broadcast table; nothing of
`x` is ever transferred to the device.

Per core:
  1. DMA a host-replicated [128, 2048] f32 PE block (1 MiB; each
     partition holds two copies of the flattened 2*pe row) into SBUF.
  2. One giant DMA on the SP HWDGE ring whose source AP has a step-0
     (broadcast) middle dim: each partition re-reads its 8 KiB segment
     30x while writing contiguous spans of the output. 8 KiB
     descriptors keep each SDMA engine at ~26 GB/s (port ceiling 27.2),
     ~400 GB/s aggregate per core.
  3. A small HBM->HBM head-start DMA on the ACT ring covers the last 2
     of the 32 output segments with no data dependency (it reads the pe
     table straight from DRAM), soaking up the otherwise-dead window
     while the SBUF input loads.

Measured on 8 axon trn2 cores: ~93.5 us HW exec in clean runs; the
chip-level HBM stacks (each shared by a pair of NCs) are the binding
resource, and inter-core interference adds run-to-run variance.
"""

import numpy as np

N_CORES = 8
BATCH = 65536
SEQ = 64
PATCH = 16
ROW = SEQ * PATCH                # 1024 f32 per batch element
SHARD = BATCH // N_CORES         # 8192 batch rows per core

DUP = 2                          # copies of the row per SBUF partition
SEG = DUP * ROW                  # 2048 f32 = 8 KiB contiguous per descriptor
SEG_REPS = SHARD // (128 * DUP)  # 32 segment repeats per partition
K_HBM = 2                        # trailing segments covered by the head-start

_CACHE = {}


def _pe2_block() -> np.ndarray:
    """[128, SEG] f32: each partition holds DUP copies of 2*pe flattened.

    Computed with jax.numpy in f32 to match the reference bit-for-bit.
    """
    import jax
    import jax.numpy as jnp

    with jax.default_device(jax.devices("cpu")[0]):
        i = jnp.arange(SEQ, dtype=jnp.float32)[:, None]
        j = jnp.arange(PATCH, dtype=jnp.float32)[None, :]
        div = i / jnp.power(jnp.float32(10000.0), j / PATCH * 2.0)
        even = (jnp.arange(PATCH) % 2 == 0)[None, :]
        pe = jnp.where(even, jnp.sin(div), jnp.cos(div))
        row = np.asarray((2.0 * pe).astype(jnp.float32)).reshape(1, ROW)
    return np.broadcast_to(np.tile(row, (1, DUP)), (128, SEG)).copy()


def _build():
    from concourse import bacc, mybir

    nc = bacc.Bacc(
        "TRN2",
        target_bir_lowering=False,
        debug=False,
        enable_partition_id=False,
        monotonic_sem_count=0,
    )
    pe_in = nc.dram_tensor(
        "pe", [128, SEG], mybir.dt.float32, kind="ExternalInput"
    ).ap()
    out = nc.dram_tensor(
        "out", [SHARD, ROW], mybir.dt.float32, kind="ExternalOutput"
    ).ap()

    with (
        nc.sbuf_tensor([128, SEG], mybir.dt.float32) as t,
        nc.semaphore() as dma_sem,
        nc.Block() as block,
    ):
        # Output viewed as [128 partition-blocks, SEG_REPS segments, SEG f32].
        dst = out.rearrange("(p r q) f -> p r (q f)", p=128, q=DUP)
        kb = SEG_REPS - K_HBM
        src_sbuf = t[:].unsqueeze(1).broadcast_to([128, kb, SEG])
        src_hbm = pe_in.unsqueeze(1).broadcast_to([128, K_HBM, SEG])

        # Input split into quarters so the SDMA engines round-robin to the
        # ACT-ring head-start between input sub-DMAs instead of draining
        # the whole input first.
        IN_SPLIT = 4
        q = SEG // IN_SPLIT

        @block.sync
        def _(sync):
            for i in range(IN_SPLIT):
                sync.dma_start(
                    t[:, i * q : (i + 1) * q], pe_in[:, i * q : (i + 1) * q]
                ).then_inc(dma_sem, 16)
            sync.wait_ge(dma_sem, 16 * IN_SPLIT)
            sync.dma_start(dst[:, :kb], src_sbuf).then_inc(dma_sem, 16)
            sync.wait_ge(dma_sem, 16 * (IN_SPLIT + 2))

        @block.scalar
        def _(scalar):
            # No data dependency: source is the pe table in DRAM itself.
            scalar.dma_start(dst[:, kb:], src_hbm).then_inc(dma_sem, 16)
            scalar.wait_ge(dma_sem, 16 * (IN_SPLIT + 2))

    nc.compile()
    return nc


def _get_nc():
    if "nc" not in _CACHE:
        _CACHE["nc"] = _build()
    return _CACHE["nc"]


def run_on_device(trace: bool = False, **kwargs):
    """Compile + run the SPMD kernel on all 8 cores; returns BassKernelResults."""
    from concourse.bass_utils import run_bass_kernel_spmd

    nc = _get_nc()
    block = _pe2_block()
    in_maps = [{"pe": block} for _ in range(N_CORES)]
    return run_bass_kernel_spmd(
        nc, in_maps, core_ids=list(range(N_CORES)), trace=trace, **kwargs
    )


def kernel(**inputs: np.ndarray) -> np.ndarray:
    x = inputs["x"]
    assert x.shape == (BATCH, SEQ, PATCH), x.shape
    res = run_on_device()
    shards = [r["out"].reshape(SHARD, SEQ, PATCH) for r in res.results]
    return np.concatenate(shards, axis=0).astype(np.float32)


# revision 25
# speedup vs baseline: 1.0429x; 1.0429x over previous
"""Trainium2 Bass kernel for nn_PositionalEncoding.

The reference module's realized math discards the input entirely:
    out = broadcast_to(2 * pe_table, x.shape)   # pe_table is [64, 16]
so the kernel is pure output-bandwidth bound. Each of the 8 cores writes
its (8192, 64, 16) f32 shard (32 MiB) of the broadcast table; nothing of
`x` is ever transferred to the device.

Per core:
  1. DMA a host-replicated [128, 2048] f32 PE block (1 MiB; each
     partition holds two copies of the flattened 2*pe row) into SBUF.
  2. One giant DMA on the SP HWDGE ring whose source AP has a step-0
     (broadcast) middle dim: each partition re-reads its 8 KiB segment
     30x while writing contiguous spans of the output. 8 KiB
     descriptors keep each SDMA engine at ~26 GB/s (port ceiling 27.2),
     ~400 GB/s aggregate per core.
  3. A small HBM->HBM head-start DMA on the ACT ring covers the last 2
     of the 32 output segments with no data dependency (it reads the pe
     table straight from DRAM), soaking up the otherwise-dead window
     while the SBUF input loads.

Measured on 8 axon trn2 cores: ~93.5 us HW exec in clean runs; the
chip-level HBM stacks (each shared by a pair of NCs) are the binding
resource, and inter-core interference adds run-to-run variance.
"""

import numpy as np

N_CORES = 8
BATCH = 65536
SEQ = 64
PATCH = 16
ROW = SEQ * PATCH                # 1024 f32 per batch element
SHARD = BATCH // N_CORES         # 8192 batch rows per core

DUP = 2                          # copies of the row per SBUF partition
SEG = DUP * ROW                  # 2048 f32 = 8 KiB contiguous per descriptor
SEG_REPS = SHARD // (128 * DUP)  # 32 segment repeats per partition
K_HBM = 2                        # trailing segments covered by the head-start

_CACHE = {}


def _pe2_block() -> np.ndarray:
    """[128, SEG] f32: each partition holds DUP copies of 2*pe flattened.

    Computed with jax.numpy in f32 to match the reference bit-for-bit.
    """
    import jax
    import jax.numpy as jnp

    with jax.default_device(jax.devices("cpu")[0]):
        i = jnp.arange(SEQ, dtype=jnp.float32)[:, None]
        j = jnp.arange(PATCH, dtype=jnp.float32)[None, :]
        div = i / jnp.power(jnp.float32(10000.0), j / PATCH * 2.0)
        even = (jnp.arange(PATCH) % 2 == 0)[None, :]
        pe = jnp.where(even, jnp.sin(div), jnp.cos(div))
        row = np.asarray((2.0 * pe).astype(jnp.float32)).reshape(1, ROW)
    return np.broadcast_to(np.tile(row, (1, DUP)), (128, SEG)).copy()


def _build():
    from concourse import bacc, mybir

    nc = bacc.Bacc(
        "TRN2",
        target_bir_lowering=False,
        debug=False,
        enable_partition_id=False,
        monotonic_sem_count=0,
    )
    pe_in = nc.dram_tensor(
        "pe", [128, SEG], mybir.dt.float32, kind="ExternalInput"
    ).ap()
    out = nc.dram_tensor(
        "out", [SHARD, ROW], mybir.dt.float32, kind="ExternalOutput"
    ).ap()

    with (
        nc.sbuf_tensor([128, SEG], mybir.dt.float32) as t,
        nc.semaphore() as dma_sem,
        nc.Block() as block,
    ):
        # Output viewed as [128 partition-blocks, SEG_REPS segments, SEG f32].
        dst = out.rearrange("(p r q) f -> p r (q f)", p=128, q=DUP)
        kb = SEG_REPS - K_HBM
        src_sbuf = t[:].unsqueeze(1).broadcast_to([128, kb, SEG])
        src_hbm = pe_in.unsqueeze(1).broadcast_to([128, K_HBM, SEG])

        @block.sync
        def _(sync):
            sync.dma_start(t[:], pe_in).then_inc(dma_sem, 16)
            sync.wait_ge(dma_sem, 16)
            sync.dma_start(dst[:, :kb], src_sbuf).then_inc(dma_sem, 16)
            sync.wait_ge(dma_sem, 48)

        @block.scalar
        def _(scalar):
            # No data dependency: source is the pe table in DRAM itself.
            scalar.dma_start(dst[:, kb:], src_hbm).then_inc(dma_sem, 16)
            scalar.wait_ge(dma_sem, 48)

    nc.compile()
    return nc


def _get_nc():
    if "nc" not in _CACHE:
        _CACHE["nc"] = _build()
    return _CACHE["nc"]


def run_on_device(trace: bool = False, **kwargs):
    """Compile + run the SPMD kernel on all 8 cores; returns BassKernelResults."""
    from concourse.bass_utils import run_bass_kernel_spmd

    nc = _get_nc()
    block = _pe2_block()
    in_maps = [{"pe": block} for _ in range(N_CORES)]
    return run_bass_kernel_spmd(
        nc, in_maps, core_ids=list(range(N_CORES)), trace=trace, **kwargs
    )


def kernel(**inputs: np.ndarray) -> np.ndarray:
    x = inputs["x"]
    assert x.shape == (BATCH, SEQ, PATCH), x.shape
    res = run_on_device()
    shards = [r["out"].reshape(SHARD, SEQ, PATCH) for r in res.results]
    return np.concatenate(shards, axis=0).astype(np.float32)
